# revision 4
# baseline (speedup 1.0000x reference)
"""Trainium2 Bass kernel for nn_MultiHeadAttention_67250597920960 (v3).

GQA attention block: q/k/v/gate projections, QK RMS-norm, RoPE, non-causal
SDPA, sigmoid gate, output projection.

Sharding: 8 cores = (batch b in {0,1}) x (kv-head group g in {0..3}).
Each core handles one batch element and one kv head (= 4 q heads) and
produces a PARTIAL output [T, C]; host sums the 4 group partials per batch.

All-bf16 datapath (fp32 psum accumulation). Key structure:
  - x host-transposed AND host-tiled -> no on-chip x transposes; chunk 0
    loads in 4 pieces interleaved with 4 wqkv pieces on the serialized
    DMA-engine pool so the first matmuls start ~2.5us in.
  - phase A per 256-token chunk: qkv -> RMS-norm (Square on ACT, reduce +
    rsqrt-via-Sqrt+recip) -> RoPE (q on DVE, k on GPSIMD; shared cos|sin
    table when the q/k norm weights are half-uniform, else folded 4-block
    tables) -> deferred PE transpose of q/k (copies out on DVE).
    Gate projections trail two chunks so wgate's DMA can come after the
    chunk-0 critical loads; sigmoids batched after the last gate (ACT stays
    on the Sqrt/Square/Copy table set all phase -> ~3 table loads total).
  - phase B per (512-token chunk c2, head): scores_T = kT.T @ qT (PE),
    wide exp ([128,1024], ACT, bf16 out), yT += v.T @ expT (PE); softmax
    denominator via bf16 DVE adds (2x packed mode) + GPSIMD
    partition_all_reduce (no PE colsum, no DRAM broadcast); recip + gate
    muls on DVE.
  - phase C (out proj) interleaved into phase B one c2 behind, as PE
    filler for the ACT-bound softmax pipeline; PSUM->SBUF copies on DVE;
    stores alternate SP/ACT DMA queues.
"""

import math
import numpy as np

# ---- problem constants (hardcoded per spec) ----
B, T, C = 2, 2048, 2048
NH, NKV, D = 16, 4, 128
HG = NH // NKV          # q heads per core = 4
GD = HG * D             # 512
P = 128
TT_N = T // P           # 16 token tiles
CT_N = C // P           # 16 channel tiles
N_CORES = 8
RMS_EPS = 1e-6
SCALE = 1.0 / math.sqrt(D)

TCH = 256               # phase A token chunk
NCH = T // TCH          # 8 chunks
TC2 = 512               # phase B token chunk
NC2 = T // TC2          # 4 chunks
GATE_LAG = 2            # gate projections trail qkv by this many chunks

DT_MODE = "bf16"


def _build_nc(dt_mode="bf16", rope_blocks=2, debug_taps=False):
    import concourse.bacc as bacc
    import concourse.mybir as mybir
    import concourse.tile as tile
    from concourse import bass_isa

    fp32 = mybir.dt.float32
    DT = mybir.dt.bfloat16
    AF = mybir.ActivationFunctionType
    RW = 64 * rope_blocks           # rope table width per token tile

    nc = bacc.Bacc("TRN2", target_bir_lowering=False, debug=False,
                   enable_asserts=False)

    xt_d = nc.dram_tensor("xt", [NCH * P, CT_N * TCH], DT,
                          kind="ExternalInput").ap()
    ident_d = nc.dram_tensor("ident", [P, P], DT, kind="ExternalInput").ap()
    wqkv_d = nc.dram_tensor("wqkv", [P, CT_N * (GD + 2 * D)], DT,
                            kind="ExternalInput").ap()
    wgate_d = nc.dram_tensor("wgate", [P, CT_N * GD], DT,
                             kind="ExternalInput").ap()
    wproj_d = nc.dram_tensor("wproj", [P, HG * C], DT,
                             kind="ExternalInput").ap()
    ropeq_d = nc.dram_tensor("ropeq", [P, TT_N * RW], DT,
                             kind="ExternalInput").ap()
    ropek_d = nc.dram_tensor("ropek", [P, TT_N * RW], DT,
                             kind="ExternalInput").ap()
    out_d = nc.dram_tensor("out", [T, C], fp32, kind="ExternalOutput").ap()
    if debug_taps:
        dbg = {
            "dbg_qT": nc.dram_tensor("dbg_qT", [P, HG * T], DT,
                                     kind="ExternalOutput").ap(),
            "dbg_kT": nc.dram_tensor("dbg_kT", [P, T], DT,
                                     kind="ExternalOutput").ap(),
            "dbg_v": nc.dram_tensor("dbg_v", [P, TT_N * P], DT,
                                    kind="ExternalOutput").ap(),
            "dbg_gate": nc.dram_tensor("dbg_gate", [P, HG * T], DT,
                                       kind="ExternalOutput").ap(),
            "dbg_ygT": nc.dram_tensor("dbg_ygT", [P, HG * T], DT,
                                      kind="ExternalOutput").ap(),
        }

    with tile.TileContext(nc) as tc:
        with tc.tile_pool(name="persist", bufs=1) as persist:
            ident = persist.tile([P, P], DT, tag="ident")
            eps_t = persist.tile([P, 1], fp32, tag="eps")
            nc.vector.memset(eps_t, RMS_EPS)

            qT_sb = persist.tile([P, HG, T], DT, tag="qT")
            kT_sb = persist.tile([P, T], DT, tag="kT")
            v_sb = persist.tile([P, TT_N, P], DT, tag="v")
            gate_sb = persist.tile([P, HG, T], DT, tag="gate")
            rope_q = persist.tile([P, TT_N, RW], DT, tag="ropeq")
            rope_k = persist.tile([P, TT_N, RW], DT, tag="ropek")
            wproj_sb = persist.tile([P, HG, C], DT, tag="wproj")

            # rope slice offsets: [cosA|sinB] shared (2-block) or
            # [cosA|sinB|sinC|cosD] folded (4-block)
            if rope_blocks == 2:
                oA, oB, oC, oD = 0, 64, 64, 0
            else:
                oA, oB, oC, oD = 0, 64, 128, 192

            # ---------------- Phase A ----------------
            with tc.tile_pool(name="wA", bufs=1) as wA, \
                 tc.tile_pool(name="xT", bufs=1 + GATE_LAG) as xTp, \
                 tc.tile_pool(name="scrA", bufs=2) as scrA, \
                 tc.tile_pool(name="qrp", bufs=4) as qrp, \
                 tc.tile_pool(name="psQKV", bufs=2, space="PSUM") as psQKV, \
                 tc.tile_pool(name="psG", bufs=2, space="PSUM") as psG, \
                 tc.tile_pool(name="psT", bufs=2, space="PSUM") as psT:

                # DMA emission order == DMA_ENGINES service order.
                # Critical first: x chunk 0 pieces (SP) interleaved with
                # wqkv pieces (ACT); then rope tables (needed ~8us), then
                # wgate (gates trail 2 chunks -> needed ~35us), ident last.
                xT_tiles = [None] * NCH
                xT0 = xTp.tile([P, CT_N, TCH], DT, tag="xT")
                xT_tiles[0] = xT0
                wqkv_sb = wA.tile([P, CT_N, GD + 2 * D], DT, tag="wqkv")
                # ropeq leads: chunk 0's q-rope feeds the DVE queue, and
                # everything later on DVE head-of-line blocks behind it
                nc.sync.dma_start(out=rope_q, in_=ropeq_d)
                for i in range(8):
                    nc.sync.dma_start(
                        out=xT_tiles[0][:, 2 * i:2 * (i + 1), :],
                        in_=xt_d[0:P, i * 512:(i + 1) * 512])
                    nc.scalar.dma_start(
                        out=wqkv_sb[:, 2 * i:2 * (i + 1), :],
                        in_=wqkv_d[:, i * 2 * 768:(i + 1) * 2 * 768])
                nc.scalar.dma_start(out=rope_k, in_=ropek_d)
                wgate_sb = wA.tile([P, CT_N, GD], DT, tag="wgate")
                with tc.tile_wait_until(0.012):
                    nc.scalar.dma_start(out=wgate_sb, in_=wgate_d)
                nc.gpsimd.dma_start(out=ident, in_=ident_d)

                def gate_chunk(ch, fuse_sigmoid=False):
                    xT_sb = xT_tiles[ch]
                    for j in range(HG):
                        g_ps = psG.tile([P, TCH], fp32, tag="gps")
                        for ct in range(CT_N):
                            nc.tensor.matmul(
                                g_ps,
                                wgate_sb[:, ct, j * P:(j + 1) * P],
                                xT_sb[:, ct, :],
                                start=(ct == 0), stop=(ct == CT_N - 1))
                        gslot = gate_sb[:, j, ch * TCH:(ch + 1) * TCH]
                        if fuse_sigmoid:
                            # trailing chunks: sigmoid table already loaded
                            nc.scalar.activation(gslot, g_ps, AF.Sigmoid)
                        else:
                            # pre-sigmoid gate parked in SBUF; sigmoid
                            # batched later to avoid act-table churn
                            with nc.allow_low_precision(reason="gate bf16"):
                                nc.vector.tensor_copy(gslot, g_ps)

                def qkT_flush(pch, pqr):
                    # transpose prior chunk's q heads and k into qT/kT [d,t]
                    for ti in range(TCH // P):
                        tt = pch * (TCH // P) + ti
                        qr = pqr[ti]
                        tq_ps = psT.tile([P, 640], DT, tag="tp")
                        for h in range(HG + 1):
                            nc.tensor.transpose(
                                tq_ps[:, h * P:(h + 1) * P],
                                qr[:, h * P:(h + 1) * P], ident)
                        nc.vector.tensor_copy(
                            qT_sb[:, :, tt * P:(tt + 1) * P],
                            tq_ps[:, 0:512].rearrange("p (h t) -> p h t",
                                                      t=P))
                        nc.vector.tensor_copy(kT_sb[:, tt * P:(tt + 1) * P],
                                              tq_ps[:, 512:640])

                pending_qkT = None
                for ch in range(NCH):
                    if ch > 0:
                        xTn = xTp.tile([P, CT_N, TCH], DT, tag="xT")
                        xT_tiles[ch] = xTn
                        # waits keep xt1-3 from flooding the DMA line
                        # ahead of the chunk-0-critical weight pieces
                        with tc.tile_wait_until(0.003 * ch, enable=ch <= 3):
                            nc.sync.dma_start(
                                out=xT_tiles[ch],
                                in_=xt_d[ch * P:(ch + 1) * P, :])
                    xT_sb = xT_tiles[ch]

                    if pending_qkT is not None:
                        qkT_flush(*pending_qkT)
                    if ch >= GATE_LAG:
                        gate_chunk(ch - GATE_LAG)

                    qr_tiles = [None] * (TCH // P)
                    # chunk 0 runs ct-major across its token tiles so each
                    # weight DMA piece is fully consumed as it lands; later
                    # chunks are tt-major (weights resident, psum bufs=2)
                    qkv_pss = []
                    for ti in range(TCH // P):
                        qkv_ps = psQKV.tile([P, GD + 2 * D], fp32, tag="qkv")
                        qkv_pss.append(qkv_ps)
                        if ch > 0:
                            for ct in range(CT_N):
                                nc.tensor.matmul(
                                    qkv_ps[:, 0:512],
                                    xT_sb[:, ct, ti * P:(ti + 1) * P],
                                    wqkv_sb[:, ct, 0:512],
                                    start=(ct == 0), stop=(ct == CT_N - 1))
                            for ct in range(CT_N):
                                nc.tensor.matmul(
                                    qkv_ps[:, 512:768],
                                    xT_sb[:, ct, ti * P:(ti + 1) * P],
                                    wqkv_sb[:, ct, 512:768],
                                    start=(ct == 0), stop=(ct == CT_N - 1))
                    if ch == 0:
                        for ct in range(CT_N):
                            for ti in range(TCH // P):
                                nc.tensor.matmul(
                                    qkv_pss[ti][:, 0:512],
                                    xT_sb[:, ct, ti * P:(ti + 1) * P],
                                    wqkv_sb[:, ct, 0:512],
                                    start=(ct == 0), stop=(ct == CT_N - 1))
                                nc.tensor.matmul(
                                    qkv_pss[ti][:, 512:768],
                                    xT_sb[:, ct, ti * P:(ti + 1) * P],
                                    wqkv_sb[:, ct, 512:768],
                                    start=(ct == 0), stop=(ct == CT_N - 1))
                    for ti in range(TCH // P):
                        tt = ch * (TCH // P) + ti
                        qkv_ps = qkv_pss[ti]

                        # RMS norm over d for q (4 heads) and k
                        sq = scrA.tile([P, 640], DT, tag="sq")
                        nc.scalar.activation(sq, qkv_ps[:, 0:640], AF.Square)
                        ssum = scrA.tile([P, 5], fp32, tag="ssum")
                        nc.vector.reduce_sum(
                            ssum, sq.rearrange("p (h d) -> p h d", d=D),
                            axis=mybir.AxisListType.X)
                        rstd = scrA.tile([P, 5], fp32, tag="rstd")
                        nc.scalar.activation(rstd, ssum, AF.Sqrt,
                                             bias=eps_t, scale=1.0 / D)
                        nc.vector.reciprocal(rstd, rstd)
                        last_rstd = rstd
                        qn = scrA.tile([P, 640], DT, tag="qn")
                        for hh in range(5):
                            nc.vector.tensor_scalar_mul(
                                qn[:, hh * D:(hh + 1) * D],
                                qkv_ps[:, hh * D:(hh + 1) * D],
                                rstd[:, hh:hh + 1])
                        # v: straight copy out of psum (ACT, table-neutral)
                        nc.scalar.copy(out=v_sb[:, tt, :],
                                       in_=qkv_ps[:, 640:768])

                        # RoPE: y1 = x1*cosA - x2*sinB; y2 = x1*sinC+x2*cosD
                        rq = rope_q[:, tt, :]
                        rk = rope_k[:, tt, :]
                        qr = qrp.tile([P, 640], DT, tag="qr")
                        s1 = scrA.tile([P, HG, 64], DT, tag="s1")
                        s2 = scrA.tile([P, HG, 64], DT, tag="s2")
                        qn3 = qn[:, 0:512].rearrange("p (h d) -> p h d", d=D)
                        qr3 = qr[:, 0:512].rearrange("p (h d) -> p h d", d=D)

                        def bcast4(ap):
                            return ap.unsqueeze(1).to_broadcast((P, HG, 64))

                        nc.vector.tensor_mul(s1, qn3[:, :, 0:64],
                                             bcast4(rq[:, oA:oA + 64]))
                        nc.vector.tensor_mul(s2, qn3[:, :, 64:128],
                                             bcast4(rq[:, oB:oB + 64]))
                        nc.vector.tensor_sub(qr3[:, :, 0:64], s1, s2)
                        nc.vector.tensor_mul(s1, qn3[:, :, 0:64],
                                             bcast4(rq[:, oC:oC + 64]))
                        nc.vector.tensor_mul(s2, qn3[:, :, 64:128],
                                             bcast4(rq[:, oD:oD + 64]))
                        nc.vector.tensor_add(qr3[:, :, 64:128], s1, s2)
                        # k rope on GPSIMD (parallel with q rope on DVE)
                        sk1 = scrA.tile([P, 64], DT, tag="sk1")
                        sk2 = scrA.tile([P, 64], DT, tag="sk2")
                        nc.gpsimd.tensor_mul(sk1, qn[:, 512:576],
                                             rk[:, oA:oA + 64])
                        nc.gpsimd.tensor_mul(sk2, qn[:, 576:640],
                                             rk[:, oB:oB + 64])
                        nc.gpsimd.tensor_sub(qr[:, 512:576], sk1, sk2)
                        nc.gpsimd.tensor_mul(sk1, qn[:, 512:576],
                                             rk[:, oC:oC + 64])
                        nc.gpsimd.tensor_mul(sk2, qn[:, 576:640],
                                             rk[:, oD:oD + 64])
                        nc.gpsimd.tensor_add(qr[:, 576:640], sk1, sk2)
                        qr_tiles[ti] = qr

                    pending_qkT = (ch, qr_tiles)

                # batched sigmoid for the non-trailing chunks (their gates
                # are long done -- runs on ACT while PE does the trailing
                # gate matmuls), then trailing gates with fused sigmoid,
                # then the last chunk's q/k transposes
                lead = NCH - GATE_LAG
                zb = persist.tile([P, 1], fp32, tag="zb")
                nc.vector.tensor_sub(zb, last_rstd[:, 0:1],
                                     last_rstd[:, 0:1])
                # first trailing gate with fused sigmoid (loads the sigmoid
                # table); the batched sigmoids for the lead chunks are each
                # pinned behind this gate's own sigmoid via their bias so
                # they can't run before it (table churn) nor hog ACT before
                # the later trailing-gate psum copies
                gate_chunk(lead, fuse_sigmoid=True)
                for j in range(HG):
                    zbj = scrA.tile([P, 1], fp32, tag="zbj")
                    gsl = gate_sb[:, j, lead * TCH:lead * TCH + 1]
                    nc.vector.tensor_sub(zbj, gsl, gsl)
                    gsig = scrA.tile([P, lead * TCH], DT, tag="gsig")
                    nc.scalar.activation(gsig, gate_sb[:, j, 0:lead * TCH],
                                         AF.Sigmoid, bias=zbj)
                    nc.vector.tensor_copy(gate_sb[:, j, 0:lead * TCH], gsig)
                for ch in range(lead + 1, NCH):
                    gate_chunk(ch, fuse_sigmoid=True)
                # preload the Exp act table behind the last trailing gate
                zbe = scrA.tile([P, 1], fp32, tag="zbe")
                gsl7 = gate_sb[:, HG - 1, T - 1:T]
                nc.vector.tensor_sub(zbe, gsl7, gsl7)
                ewarm = scrA.tile([P, 1], fp32, tag="ewarm")
                nc.scalar.activation(ewarm, zbe, AF.Exp)
                if pending_qkT is not None:
                    qkT_flush(*pending_qkT)

            # ------- Phase B + C (proj interleaved as PE filler) -------
            with tc.tile_pool(name="ygT", bufs=1) as ygTp:
                ygT_sb = ygTp.tile([P, HG, T], DT, tag="ygT")
                # wproj prefetch overlaps late phase A / early phase B
                with tc.tile_wait_until(0.10):
                    nc.gpsimd.dma_start(out=wproj_sb, in_=wproj_d)

                with tc.tile_pool(name="expB", bufs=3) as expB, \
                     tc.tile_pool(name="gB", bufs=2) as gB, \
                     tc.tile_pool(name="ost", bufs=4) as ostp2, \
                     tc.tile_pool(name="psSC", bufs=2, space="PSUM") as psSC, \
                     tc.tile_pool(name="psY", bufs=2, space="PSUM") as psY, \
                     tc.tile_pool(name="psC", bufs=2, space="PSUM") as psC:

                    def proj_unit(tt, et, act_copy=False):
                        # phase-C unit: out[tt-tile, et*512:...] -- PE filler
                        # for the ACT-bound softmax pipeline
                        o_ps = psC.tile([P, 512], fp32, tag="ops")
                        for hd in range(HG):
                            nc.tensor.matmul(
                                o_ps,
                                ygT_sb[:, hd, tt * P:(tt + 1) * P],
                                wproj_sb[:, hd, et * 512:(et + 1) * 512],
                                start=(hd == 0), stop=(hd == HG - 1))
                        o_sb = ostp2.tile([P, 512], fp32, tag="osb")
                        if act_copy:
                            nc.scalar.copy(o_sb, o_ps)
                        else:
                            nc.vector.tensor_copy(o_sb, o_ps)
                        eng = nc.sync if et % 2 == 0 else nc.scalar
                        eng.dma_start(
                            out=out_d[tt * P:(tt + 1) * P,
                                      et * 512:(et + 1) * 512],
                            in_=o_sb)

                    def attn_head(c2, h):
                        tsl = slice(c2 * TC2, (c2 + 1) * TC2)
                        yT_ps = psY.tile([P, TC2], fp32, tag="yT")
                        esum = gB.tile([P, 2 * TC2], DT, tag="esum")

                        def sc_pair(stp):
                            sc_ps = psSC.tile([P, 2 * TC2], fp32, tag="sc")
                            for k in range(2):
                                nc.tensor.matmul(
                                    sc_ps[:, k * TC2:(k + 1) * TC2],
                                    kT_sb[:, (2 * stp + k) * P:
                                          (2 * stp + k + 1) * P],
                                    qT_sb[:, h, tsl],
                                    start=True, stop=True)
                            e_sb = expB.tile([P, 2 * TC2], DT, tag="exp")
                            nc.scalar.activation(e_sb, sc_ps, AF.Exp,
                                                 scale=SCALE)
                            return e_sb

                        def yc_pair(stp, e_sb):
                            first, last = stp == 0, stp == TT_N // 2 - 1
                            for k in range(2):
                                nc.tensor.matmul(
                                    yT_ps, v_sb[:, 2 * stp + k, :],
                                    e_sb[:, k * TC2:(k + 1) * TC2],
                                    start=(first and k == 0),
                                    stop=(last and k == 1))

                        def add_step(stp, e_sb):
                            if stp == 0:
                                nc.vector.tensor_copy(esum, e_sb)
                            else:
                                nc.vector.tensor_add(esum, esum, e_sb)

                        # software pipeline: scores(p+1) before y(p)
                        prev = sc_pair(0)
                        for stp in range(1, TT_N // 2):
                            cur = sc_pair(stp)
                            yc_pair(stp - 1, prev)
                            add_step(stp - 1, prev)
                            prev = cur
                        yc_pair(TT_N // 2 - 1, prev)
                        add_step(TT_N // 2 - 1, prev)

                        # softmax denominator: fold halves on DVE, then a
                        # GPSIMD all-reduce leaves column sums in every
                        # partition -- no PE colsum, no DRAM broadcast
                        esum2 = gB.tile([P, TC2], DT, tag="esum2")
                        nc.vector.tensor_add(esum2, esum[:, 0:TC2],
                                             esum[:, TC2:2 * TC2])
                        den = gB.tile([P, TC2], fp32, tag="den")
                        nc.gpsimd.partition_all_reduce(
                            den, esum2, channels=P,
                            reduce_op=bass_isa.ReduceOp.add)
                        rc_sb = gB.tile([P, TC2], fp32, tag="rc")
                        nc.vector.reciprocal(rc_sb, den)
                        gsc_sb = gB.tile([P, TC2], fp32, tag="gsc")
                        nc.vector.tensor_mul(gsc_sb, gate_sb[:, h, tsl],
                                             rc_sb)
                        nc.vector.tensor_mul(ygT_sb[:, h, tsl], yT_ps,
                                             gsc_sb)

                    def proj_c2(c2):
                        # endgame: ACT is idle after the last exp -- split
                        # the copies across ACT and DVE to shorten the tail
                        for ti in range(TC2 // P):
                            tt = c2 * (TC2 // P) + ti
                            for et in range(C // 512):
                                proj_unit(tt, et, act_copy=(et % 2 == 1))

                    # proj units trail attention by one c2 so they fill the
                    # ACT-bound stretches of the NEXT chunk's softmax
                    for c2 in range(NC2):
                        for h in range(HG):
                            attn_head(c2, h)
                            if c2 > 0:
                                ti = h
                                tt = (c2 - 1) * (TC2 // P) + ti
                                for et in range(C // 512):
                                    proj_unit(tt, et)
                    proj_c2(NC2 - 1)
                    if debug_taps:
                        nc.sync.dma_start(out=dbg["dbg_qT"], in_=qT_sb)
                        nc.sync.dma_start(out=dbg["dbg_kT"], in_=kT_sb)
                        nc.sync.dma_start(out=dbg["dbg_v"], in_=v_sb)
                        nc.sync.dma_start(out=dbg["dbg_gate"], in_=gate_sb)
                        nc.scalar.dma_start(out=dbg["dbg_ygT"], in_=ygT_sb)
    nc.compile()
    return nc


def make_core_inputs(x, cos, sin, wq, wk, wv, w_gate, w_proj,
                     q_norm_w, k_norm_w, dt_mode=DT_MODE):
    """Host-side prep: per-core input dicts (pretiled, bf16)."""
    import ml_dtypes
    cdt = ml_dtypes.bfloat16

    cosf = np.asarray(cos, np.float32).reshape(T, 64)
    sinf = np.asarray(sin, np.float32).reshape(T, 64)
    qw = np.asarray(q_norm_w, np.float32)
    kw = np.asarray(k_norm_w, np.float32)

    # 2-block [cos|sin] tables when each norm-weight vector is identical
    # across its two rotary halves (covers the common all-ones case);
    # 4-block folded tables otherwise.
    if (np.array_equal(qw[:64], qw[64:]) and np.array_equal(kw[:64], kw[64:])):
        rope_blocks = 2
        ropeq = np.concatenate([cosf * qw[:64], sinf * qw[:64]], axis=1)
        ropek = np.concatenate([cosf * kw[:64], sinf * kw[:64]], axis=1)
    else:
        rope_blocks = 4
        ropeq = np.concatenate([cosf * qw[:64], sinf * qw[64:],
                                sinf * qw[:64], cosf * qw[64:]], axis=1)
        ropek = np.concatenate([cosf * kw[:64], sinf * kw[64:],
                                sinf * kw[:64], cosf * kw[64:]], axis=1)

    def tile_rows(a, n_tiles):
        # [n_tiles*P, W] -> [P, n_tiles*W] with (tile, p, w) -> (p, tile*W+w)
        w = a.shape[1]
        return np.ascontiguousarray(
            a.reshape(n_tiles, P, w).transpose(1, 0, 2).reshape(P, -1))

    ropeq_t = tile_rows(ropeq, TT_N).astype(cdt)
    ropek_t = tile_rows(ropek, TT_N).astype(cdt)

    x = np.asarray(x, np.float32)
    in_maps = []
    for core in range(N_CORES):
        b, g = core // NKV, core % NKV
        xb = x[b]                                    # [T, C]
        xt = xb.reshape(NCH, TCH, CT_N, P).transpose(0, 3, 2, 1)
        xt = np.ascontiguousarray(xt).reshape(NCH * P, CT_N * TCH)
        wqkv = np.concatenate([wq[:, g * GD:(g + 1) * GD],
                               wk[:, g * D:(g + 1) * D],
                               wv[:, g * D:(g + 1) * D]], axis=1)
        wqkv = np.asarray(wqkv, np.float32)
        wg = np.asarray(w_gate[:, g * GD:(g + 1) * GD], np.float32)
        wp = np.asarray(w_proj[g * GD:(g + 1) * GD, :], np.float32)
        in_maps.append({
            "xt": xt.astype(cdt),
            "ident": np.eye(P, dtype=np.float32).astype(cdt),
            "wqkv": tile_rows(wqkv, CT_N).astype(cdt),
            "wgate": tile_rows(wg, CT_N).astype(cdt),
            "wproj": tile_rows(wp, HG).astype(cdt),
            "ropeq": ropeq_t,
            "ropek": ropek_t,
        })
    return in_maps, rope_blocks


def kernel(x, cos, sin, wq, wk, wv, w_gate, w_proj, q_norm_w, k_norm_w):
    from concourse.bass_utils import run_bass_kernel_spmd

    x = np.asarray(x)
    in_maps, rope_blocks = make_core_inputs(
        x, cos, sin, wq, wk, wv, w_gate, w_proj, q_norm_w, k_norm_w)
    nc = _build_nc(DT_MODE, rope_blocks=rope_blocks)
    res = run_bass_kernel_spmd(nc, in_maps, list(range(N_CORES)))
    partial = np.stack([res.results[i]["out"] for i in range(N_CORES)])
    out = partial.reshape(B, NKV, T, C).sum(axis=1)
    return out.astype(np.float32)


# revision 5
# speedup vs baseline: 2.5698x; 2.5698x over previous
"""Trainium2 Bass kernel for nn_MultiHeadAttention_67250597920960 (v3).

GQA attention block: q/k/v/gate projections, QK RMS-norm, RoPE, non-causal
SDPA, sigmoid gate, output projection.

Sharding: 8 cores = (batch b in {0,1}) x (kv-head group g in {0..3}).
Each core handles one batch element and one kv head (= 4 q heads) and
produces a PARTIAL output [T, C]; host sums the 4 group partials per batch.

All-bf16 datapath (fp32 psum accumulation). Key structure:
  - x host-transposed AND host-tiled -> no on-chip x transposes; chunk 0
    loads in 4 pieces interleaved with 4 wqkv pieces on the serialized
    DMA-engine pool so the first matmuls start ~2.5us in.
  - phase A per 256-token chunk: qkv -> RMS-norm (Square on ACT, reduce +
    rsqrt-via-Sqrt+recip) -> RoPE (q on DVE, k on GPSIMD; shared cos|sin
    table when the q/k norm weights are half-uniform, else folded 4-block
    tables) -> deferred PE transpose of q/k (copies out on DVE).
    Gate projections trail two chunks so wgate's DMA can come after the
    chunk-0 critical loads; sigmoids batched after the last gate (ACT stays
    on the Sqrt/Square/Copy table set all phase -> ~3 table loads total).
  - phase B per (512-token chunk c2, head): scores_T = kT.T @ qT (PE),
    wide exp ([128,1024], ACT, bf16 out), yT += v.T @ expT (PE); softmax
    denominator via bf16 DVE adds (2x packed mode) + GPSIMD
    partition_all_reduce (no PE colsum, no DRAM broadcast); recip + gate
    muls on DVE.
  - phase C (out proj) interleaved into phase B one c2 behind, as PE
    filler for the ACT-bound softmax pipeline; PSUM->SBUF copies on DVE;
    stores alternate SP/ACT DMA queues.
"""

import math
import numpy as np

# ---- problem constants (hardcoded per spec) ----
B, T, C = 2, 2048, 2048
NH, NKV, D = 16, 4, 128
HG = NH // NKV          # q heads per core = 4
GD = HG * D             # 512
P = 128
TT_N = T // P           # 16 token tiles
CT_N = C // P           # 16 channel tiles
N_CORES = 8
RMS_EPS = 1e-6
SCALE = 1.0 / math.sqrt(D)

TCH = 256               # phase A token chunk
NCH = T // TCH          # 8 chunks
TC2 = 512               # phase B token chunk
NC2 = T // TC2          # 4 chunks
GATE_LAG = 2            # gate projections trail qkv by this many chunks

DT_MODE = "bf16"


def _build_nc(dt_mode="bf16", rope_blocks=2, debug_taps=False):
    import concourse.bacc as bacc
    import concourse.mybir as mybir
    import concourse.tile as tile
    from concourse import bass_isa

    fp32 = mybir.dt.float32
    DT = mybir.dt.bfloat16
    AF = mybir.ActivationFunctionType
    RW = 64 * rope_blocks           # rope table width per token tile

    nc = bacc.Bacc("TRN2", target_bir_lowering=False, debug=False,
                   enable_asserts=False)

    xt_d = nc.dram_tensor("xt", [NCH * P, CT_N * TCH], DT,
                          kind="ExternalInput").ap()
    ident_d = nc.dram_tensor("ident", [P, P], DT, kind="ExternalInput").ap()
    wqkv_d = nc.dram_tensor("wqkv", [P, CT_N * (GD + 2 * D)], DT,
                            kind="ExternalInput").ap()
    wgate_d = nc.dram_tensor("wgate", [P, CT_N * GD], DT,
                             kind="ExternalInput").ap()
    wproj_d = nc.dram_tensor("wproj", [P, HG * C], DT,
                             kind="ExternalInput").ap()
    ropeq_d = nc.dram_tensor("ropeq", [P, TT_N * RW], DT,
                             kind="ExternalInput").ap()
    ropek_d = nc.dram_tensor("ropek", [P, TT_N * RW], DT,
                             kind="ExternalInput").ap()
    out_d = nc.dram_tensor("out", [T, C], fp32, kind="ExternalOutput").ap()
    if debug_taps:
        dbg = {
            "dbg_qT": nc.dram_tensor("dbg_qT", [P, HG * T], DT,
                                     kind="ExternalOutput").ap(),
            "dbg_kT": nc.dram_tensor("dbg_kT", [P, T], DT,
                                     kind="ExternalOutput").ap(),
            "dbg_v": nc.dram_tensor("dbg_v", [P, TT_N * P], DT,
                                    kind="ExternalOutput").ap(),
            "dbg_gate": nc.dram_tensor("dbg_gate", [P, HG * T], DT,
                                       kind="ExternalOutput").ap(),
            "dbg_ygT": nc.dram_tensor("dbg_ygT", [P, HG * T], DT,
                                      kind="ExternalOutput").ap(),
        }

    with tile.TileContext(nc) as tc:
        with tc.tile_pool(name="persist", bufs=1) as persist:
            ident = persist.tile([P, P], DT, tag="ident")
            eps_t = persist.tile([P, 1], fp32, tag="eps")
            nc.vector.memset(eps_t, RMS_EPS)
            # first ACT op: pulls the Sqrt table in at t~1us (otherwise the
            # load lands on the critical path at the first RMS-norm)
            swarm = persist.tile([P, 1], fp32, tag="swarm")
            nc.scalar.activation(swarm, eps_t, AF.Sqrt)

            qT_sb = persist.tile([P, HG, T], DT, tag="qT")
            kT_sb = persist.tile([P, T], DT, tag="kT")
            v_sb = persist.tile([P, TT_N, P], DT, tag="v")
            gate_sb = persist.tile([P, HG, T], DT, tag="gate")
            rope_q = persist.tile([P, TT_N, RW], DT, tag="ropeq")
            rope_k = persist.tile([P, TT_N, RW], DT, tag="ropek")
            wproj_sb = persist.tile([P, HG, C], DT, tag="wproj")

            # rope slice offsets: [cosA|sinB] shared (2-block) or
            # [cosA|sinB|sinC|cosD] folded (4-block)
            if rope_blocks == 2:
                oA, oB, oC, oD = 0, 64, 64, 0
            else:
                oA, oB, oC, oD = 0, 64, 128, 192

            # ---------------- Phase A ----------------
            with tc.tile_pool(name="wA", bufs=1) as wA, \
                 tc.tile_pool(name="xT", bufs=1 + GATE_LAG) as xTp, \
                 tc.tile_pool(name="scrA", bufs=2) as scrA, \
                 tc.tile_pool(name="qrp", bufs=4) as qrp, \
                 tc.tile_pool(name="psQKV", bufs=3, space="PSUM") as psQKV, \
                 tc.tile_pool(name="psG", bufs=1, space="PSUM") as psG, \
                 tc.tile_pool(name="psT", bufs=1, space="PSUM") as psT:

                # DMA emission order == DMA_ENGINES service order.
                # Critical first: x chunk 0 pieces (SP) interleaved with
                # wqkv pieces (ACT); then rope tables (needed ~8us), then
                # wgate (gates trail 2 chunks -> needed ~35us), ident last.
                xT_tiles = [None] * NCH
                xT0 = xTp.tile([P, CT_N, TCH], DT, tag="xT")
                xT_tiles[0] = xT0
                wqkv_sb = wA.tile([P, CT_N, GD + 2 * D], DT, tag="wqkv")
                # ropeq leads: chunk 0's q-rope feeds the DVE queue, and
                # everything later on DVE head-of-line blocks behind it
                nc.sync.dma_start(out=rope_q, in_=ropeq_d)
                for i in range(8):
                    nc.sync.dma_start(
                        out=xT_tiles[0][:, 2 * i:2 * (i + 1), :],
                        in_=xt_d[0:P, i * 512:(i + 1) * 512])
                    nc.scalar.dma_start(
                        out=wqkv_sb[:, 2 * i:2 * (i + 1), :],
                        in_=wqkv_d[:, i * 2 * 768:(i + 1) * 2 * 768])
                nc.scalar.dma_start(out=rope_k, in_=ropek_d)
                wgate_sb = wA.tile([P, CT_N, GD], DT, tag="wgate")
                with tc.tile_wait_until(0.010):
                    nc.scalar.dma_start(out=wgate_sb, in_=wgate_d)
                nc.gpsimd.dma_start(out=ident, in_=ident_d)

                def gate_chunk(ch, fuse_sigmoid=False):
                    xT_sb = xT_tiles[ch]
                    for j in range(HG):
                        g_ps = psG.tile([P, TCH], fp32, tag="gps")
                        for ct in range(CT_N):
                            nc.tensor.matmul(
                                g_ps,
                                wgate_sb[:, ct, j * P:(j + 1) * P],
                                xT_sb[:, ct, :],
                                start=(ct == 0), stop=(ct == CT_N - 1))
                        gslot = gate_sb[:, j, ch * TCH:(ch + 1) * TCH]
                        if fuse_sigmoid:
                            # trailing chunks: sigmoid table already loaded
                            nc.scalar.activation(gslot, g_ps, AF.Sigmoid)
                        else:
                            # pre-sigmoid gate parked in SBUF; sigmoid
                            # batched later to avoid act-table churn
                            with nc.allow_low_precision(reason="gate bf16"):
                                nc.vector.tensor_copy(gslot, g_ps)

                def qkT_flush(pch, pqr):
                    # transpose prior chunk's q heads and k into qT/kT [d,t]
                    for ti in range(TCH // P):
                        tt = pch * (TCH // P) + ti
                        qr = pqr[ti]
                        tq_ps = psT.tile([P, 640], DT, tag="tp")
                        for h in range(HG + 1):
                            nc.tensor.transpose(
                                tq_ps[:, h * P:(h + 1) * P],
                                qr[:, h * P:(h + 1) * P], ident)
                        nc.vector.tensor_copy(
                            qT_sb[:, :, tt * P:(tt + 1) * P],
                            tq_ps[:, 0:512].rearrange("p (h t) -> p h t",
                                                      t=P))
                        nc.vector.tensor_copy(kT_sb[:, tt * P:(tt + 1) * P],
                                              tq_ps[:, 512:640])

                pending_qkT = None
                for ch in range(NCH):
                    if ch > 0:
                        xTn = xTp.tile([P, CT_N, TCH], DT, tag="xT")
                        xT_tiles[ch] = xTn
                        # waits keep xt1-3 from flooding the DMA line
                        # ahead of the chunk-0-critical weight pieces
                        with tc.tile_wait_until(0.0025 * ch, enable=ch <= 3):
                            nc.sync.dma_start(
                                out=xT_tiles[ch],
                                in_=xt_d[ch * P:(ch + 1) * P, :])
                    xT_sb = xT_tiles[ch]

                    if pending_qkT is not None:
                        qkT_flush(*pending_qkT)
                    if ch >= GATE_LAG:
                        gate_chunk(ch - GATE_LAG)

                    qr_tiles = [None] * (TCH // P)
                    # chunk 0 runs ct-major across its token tiles so each
                    # weight DMA piece is fully consumed as it lands; later
                    # chunks are tt-major (weights resident, psum bufs=2)
                    qkv_pss = []
                    for ti in range(TCH // P):
                        qkv_ps = psQKV.tile([P, GD + 2 * D], fp32, tag="qkv")
                        qkv_pss.append(qkv_ps)
                        if ch > 0:
                            for ct in range(CT_N):
                                nc.tensor.matmul(
                                    qkv_ps[:, 0:512],
                                    xT_sb[:, ct, ti * P:(ti + 1) * P],
                                    wqkv_sb[:, ct, 0:512],
                                    start=(ct == 0), stop=(ct == CT_N - 1))
                            for ct in range(CT_N):
                                nc.tensor.matmul(
                                    qkv_ps[:, 512:768],
                                    xT_sb[:, ct, ti * P:(ti + 1) * P],
                                    wqkv_sb[:, ct, 512:768],
                                    start=(ct == 0), stop=(ct == CT_N - 1))
                    if ch == 0:
                        for ct in range(CT_N):
                            for ti in range(TCH // P):
                                nc.tensor.matmul(
                                    qkv_pss[ti][:, 0:512],
                                    xT_sb[:, ct, ti * P:(ti + 1) * P],
                                    wqkv_sb[:, ct, 0:512],
                                    start=(ct == 0), stop=(ct == CT_N - 1))
                                nc.tensor.matmul(
                                    qkv_pss[ti][:, 512:768],
                                    xT_sb[:, ct, ti * P:(ti + 1) * P],
                                    wqkv_sb[:, ct, 512:768],
                                    start=(ct == 0), stop=(ct == CT_N - 1))
                    for ti in range(TCH // P):
                        tt = ch * (TCH // P) + ti
                        qkv_ps = qkv_pss[ti]

                        # RMS norm over d for q (4 heads) and k
                        sq = scrA.tile([P, 640], DT, tag="sq")
                        nc.scalar.activation(sq, qkv_ps[:, 0:640], AF.Square)
                        ssum = scrA.tile([P, 5], fp32, tag="ssum")
                        nc.vector.reduce_sum(
                            ssum, sq.rearrange("p (h d) -> p h d", d=D),
                            axis=mybir.AxisListType.X)
                        rstd = scrA.tile([P, 5], fp32, tag="rstd")
                        nc.scalar.activation(rstd, ssum, AF.Sqrt,
                                             bias=eps_t, scale=1.0 / D)
                        nc.vector.reciprocal(rstd, rstd)
                        last_rstd = rstd
                        qn = scrA.tile([P, 640], DT, tag="qn")
                        for hh in range(5):
                            nc.vector.tensor_scalar_mul(
                                qn[:, hh * D:(hh + 1) * D],
                                qkv_ps[:, hh * D:(hh + 1) * D],
                                rstd[:, hh:hh + 1])
                        # v: straight copy out of psum (ACT, table-neutral)
                        nc.scalar.copy(out=v_sb[:, tt, :],
                                       in_=qkv_ps[:, 640:768])

                        # RoPE: y1 = x1*cosA - x2*sinB; y2 = x1*sinC+x2*cosD
                        rq = rope_q[:, tt, :]
                        rk = rope_k[:, tt, :]
                        qr = qrp.tile([P, 640], DT, tag="qr")
                        s1 = scrA.tile([P, HG, 64], DT, tag="s1")
                        s2 = scrA.tile([P, HG, 64], DT, tag="s2")
                        qn3 = qn[:, 0:512].rearrange("p (h d) -> p h d", d=D)
                        qr3 = qr[:, 0:512].rearrange("p (h d) -> p h d", d=D)

                        def bcast4(ap):
                            return ap.unsqueeze(1).to_broadcast((P, HG, 64))

                        nc.vector.tensor_mul(s1, qn3[:, :, 0:64],
                                             bcast4(rq[:, oA:oA + 64]))
                        nc.vector.tensor_mul(s2, qn3[:, :, 64:128],
                                             bcast4(rq[:, oB:oB + 64]))
                        nc.vector.tensor_sub(qr3[:, :, 0:64], s1, s2)
                        nc.vector.tensor_mul(s1, qn3[:, :, 0:64],
                                             bcast4(rq[:, oC:oC + 64]))
                        nc.vector.tensor_mul(s2, qn3[:, :, 64:128],
                                             bcast4(rq[:, oD:oD + 64]))
                        nc.vector.tensor_add(qr3[:, :, 64:128], s1, s2)
                        # k rope on GPSIMD (parallel with q rope on DVE)
                        sk1 = scrA.tile([P, 64], DT, tag="sk1")
                        sk2 = scrA.tile([P, 64], DT, tag="sk2")
                        nc.gpsimd.tensor_mul(sk1, qn[:, 512:576],
                                             rk[:, oA:oA + 64])
                        nc.gpsimd.tensor_mul(sk2, qn[:, 576:640],
                                             rk[:, oB:oB + 64])
                        nc.gpsimd.tensor_sub(qr[:, 512:576], sk1, sk2)
                        nc.gpsimd.tensor_mul(sk1, qn[:, 512:576],
                                             rk[:, oC:oC + 64])
                        nc.gpsimd.tensor_mul(sk2, qn[:, 576:640],
                                             rk[:, oD:oD + 64])
                        nc.gpsimd.tensor_add(qr[:, 576:640], sk1, sk2)
                        qr_tiles[ti] = qr

                    pending_qkT = (ch, qr_tiles)

                # batched sigmoid for the non-trailing chunks (their gates
                # are long done -- runs on ACT while PE does the trailing
                # gate matmuls), then trailing gates with fused sigmoid,
                # then the last chunk's q/k transposes
                lead = NCH - GATE_LAG
                zb = persist.tile([P, 1], fp32, tag="zb")
                nc.vector.tensor_sub(zb, last_rstd[:, 0:1],
                                     last_rstd[:, 0:1])
                # first trailing gate with fused sigmoid (loads the sigmoid
                # table); the batched sigmoids for the lead chunks are each
                # pinned behind this gate's own sigmoid via their bias so
                # they can't run before it (table churn) nor hog ACT before
                # the later trailing-gate psum copies
                gate_chunk(lead, fuse_sigmoid=True)
                for j in range(HG):
                    zbj = scrA.tile([P, 1], fp32, tag="zbj")
                    gsl = gate_sb[:, j, lead * TCH:lead * TCH + 1]
                    nc.vector.tensor_sub(zbj, gsl, gsl)
                    gsig = scrA.tile([P, lead * TCH], DT, tag="gsig")
                    nc.scalar.activation(gsig, gate_sb[:, j, 0:lead * TCH],
                                         AF.Sigmoid, bias=zbj)
                    nc.vector.tensor_copy(gate_sb[:, j, 0:lead * TCH], gsig)
                for ch in range(lead + 1, NCH):
                    gate_chunk(ch, fuse_sigmoid=True)
                # preload the Exp act table behind the last trailing gate
                zbe = scrA.tile([P, 1], fp32, tag="zbe")
                gsl7 = gate_sb[:, HG - 1, T - 1:T]
                nc.vector.tensor_sub(zbe, gsl7, gsl7)
                ewarm = scrA.tile([P, 1], fp32, tag="ewarm")
                nc.scalar.activation(ewarm, zbe, AF.Exp)
                if pending_qkT is not None:
                    qkT_flush(*pending_qkT)

            # ------- Phase B + C (proj interleaved as PE filler) -------
            with tc.tile_pool(name="ygT", bufs=1) as ygTp:
                ygT_sb = ygTp.tile([P, HG, T], DT, tag="ygT")
                # wproj prefetch overlaps late phase A / early phase B
                with tc.tile_wait_until(0.10):
                    nc.gpsimd.dma_start(out=wproj_sb, in_=wproj_d)

                with tc.tile_pool(name="expB", bufs=3) as expB, \
                     tc.tile_pool(name="gB", bufs=2) as gB, \
                     tc.tile_pool(name="ost", bufs=4) as ostp2, \
                     tc.tile_pool(name="psSC", bufs=2, space="PSUM") as psSC, \
                     tc.tile_pool(name="psY", bufs=2, space="PSUM") as psY, \
                     tc.tile_pool(name="psC", bufs=2, space="PSUM") as psC:

                    def proj_unit(tt, et, act_copy=False, fine=False):
                        # phase-C unit: out[tt-tile, et*512:...] -- PE filler
                        # for the ACT-bound softmax pipeline. fine=True
                        # splits the drain into 256-wide halves (short tail).
                        o_ps = psC.tile([P, 512], fp32, tag="ops")
                        for hd in range(HG):
                            nc.tensor.matmul(
                                o_ps,
                                ygT_sb[:, hd, tt * P:(tt + 1) * P],
                                wproj_sb[:, hd, et * 512:(et + 1) * 512],
                                start=(hd == 0), stop=(hd == HG - 1))
                        o_sb = ostp2.tile([P, 512], fp32, tag="osb")
                        parts = ((0, 256), (256, 512)) if fine else ((0, 512),)
                        for pi, (a, b) in enumerate(parts):
                            if (act_copy + pi) % 2:
                                nc.scalar.copy(o_sb[:, a:b], o_ps[:, a:b])
                            else:
                                nc.vector.tensor_copy(o_sb[:, a:b],
                                                      o_ps[:, a:b])
                            eng = nc.sync if (et + pi) % 2 == 0 else nc.scalar
                            eng.dma_start(
                                out=out_d[tt * P:(tt + 1) * P,
                                          et * 512 + a:et * 512 + b],
                                in_=o_sb[:, a:b])

                    def attn_head(c2, h):
                        tsl = slice(c2 * TC2, (c2 + 1) * TC2)
                        yT_ps = psY.tile([P, TC2], fp32, tag="yT")
                        esum = gB.tile([P, 2 * TC2], DT, tag="esum")

                        def sc_pair(stp):
                            sc_ps = psSC.tile([P, 2 * TC2], fp32, tag="sc")
                            for k in range(2):
                                nc.tensor.matmul(
                                    sc_ps[:, k * TC2:(k + 1) * TC2],
                                    kT_sb[:, (2 * stp + k) * P:
                                          (2 * stp + k + 1) * P],
                                    qT_sb[:, h, tsl],
                                    start=True, stop=True)
                            e_sb = expB.tile([P, 2 * TC2], DT, tag="exp")
                            nc.scalar.activation(e_sb, sc_ps, AF.Exp,
                                                 scale=SCALE)
                            return e_sb

                        def yc_pair(stp, e_sb):
                            first, last = stp == 0, stp == TT_N // 2 - 1
                            for k in range(2):
                                nc.tensor.matmul(
                                    yT_ps, v_sb[:, 2 * stp + k, :],
                                    e_sb[:, k * TC2:(k + 1) * TC2],
                                    start=(first and k == 0),
                                    stop=(last and k == 1))

                        def add_step(stp, e_sb):
                            if stp == 0:
                                nc.vector.tensor_copy(esum, e_sb)
                            else:
                                nc.vector.tensor_add(esum, esum, e_sb)

                        # software pipeline: scores(p+1) before y(p)
                        prev = sc_pair(0)
                        for stp in range(1, TT_N // 2):
                            cur = sc_pair(stp)
                            yc_pair(stp - 1, prev)
                            add_step(stp - 1, prev)
                            prev = cur
                        yc_pair(TT_N // 2 - 1, prev)
                        add_step(TT_N // 2 - 1, prev)

                        # softmax denominator: fold halves on DVE, then a
                        # GPSIMD all-reduce leaves column sums in every
                        # partition -- no PE colsum, no DRAM broadcast
                        esum2 = gB.tile([P, TC2], DT, tag="esum2")
                        nc.vector.tensor_add(esum2, esum[:, 0:TC2],
                                             esum[:, TC2:2 * TC2])
                        den = gB.tile([P, TC2], fp32, tag="den")
                        nc.gpsimd.partition_all_reduce(
                            den, esum2, channels=P,
                            reduce_op=bass_isa.ReduceOp.add)
                        rc_sb = gB.tile([P, TC2], fp32, tag="rc")
                        nc.vector.reciprocal(rc_sb, den)
                        gsc_sb = gB.tile([P, TC2], fp32, tag="gsc")
                        nc.vector.tensor_mul(gsc_sb, gate_sb[:, h, tsl],
                                             rc_sb)
                        nc.vector.tensor_mul(ygT_sb[:, h, tsl], yT_ps,
                                             gsc_sb)

                    def proj_c2(c2):
                        # endgame: ACT is idle after the last exp -- split
                        # the copies across ACT and DVE to shorten the tail
                        for ti in range(TC2 // P):
                            tt = c2 * (TC2 // P) + ti
                            for et in range(C // 512):
                                proj_unit(tt, et, act_copy=(et % 2 == 1))

                    # proj units trail attention by one c2 so they fill the
                    # ACT-bound stretches of the NEXT chunk's softmax
                    for c2 in range(NC2):
                        for h in range(HG):
                            attn_head(c2, h)
                            if c2 > 0:
                                ti = h
                                tt = (c2 - 1) * (TC2 // P) + ti
                                for et in range(C // 512):
                                    proj_unit(tt, et)
                    proj_c2(NC2 - 1)
                    if debug_taps:
                        nc.sync.dma_start(out=dbg["dbg_qT"], in_=qT_sb)
                        nc.sync.dma_start(out=dbg["dbg_kT"], in_=kT_sb)
                        nc.sync.dma_start(out=dbg["dbg_v"], in_=v_sb)
                        nc.sync.dma_start(out=dbg["dbg_gate"], in_=gate_sb)
                        nc.scalar.dma_start(out=dbg["dbg_ygT"], in_=ygT_sb)
    nc.compile()
    return nc


def make_core_inputs(x, cos, sin, wq, wk, wv, w_gate, w_proj,
                     q_norm_w, k_norm_w, dt_mode=DT_MODE):
    """Host-side prep: per-core input dicts (pretiled, bf16)."""
    import ml_dtypes
    cdt = ml_dtypes.bfloat16

    cosf = np.asarray(cos, np.float32).reshape(T, 64)
    sinf = np.asarray(sin, np.float32).reshape(T, 64)
    qw = np.asarray(q_norm_w, np.float32)
    kw = np.asarray(k_norm_w, np.float32)

    # 2-block [cos|sin] tables when each norm-weight vector is identical
    # across its two rotary halves (covers the common all-ones case);
    # 4-block folded tables otherwise.
    if (np.array_equal(qw[:64], qw[64:]) and np.array_equal(kw[:64], kw[64:])):
        rope_blocks = 2
        ropeq = np.concatenate([cosf * qw[:64], sinf * qw[:64]], axis=1)
        ropek = np.concatenate([cosf * kw[:64], sinf * kw[:64]], axis=1)
    else:
        rope_blocks = 4
        ropeq = np.concatenate([cosf * qw[:64], sinf * qw[64:],
                                sinf * qw[:64], cosf * qw[64:]], axis=1)
        ropek = np.concatenate([cosf * kw[:64], sinf * kw[64:],
                                sinf * kw[:64], cosf * kw[64:]], axis=1)

    def tile_rows(a, n_tiles):
        # [n_tiles*P, W] -> [P, n_tiles*W] with (tile, p, w) -> (p, tile*W+w)
        w = a.shape[1]
        return np.ascontiguousarray(
            a.reshape(n_tiles, P, w).transpose(1, 0, 2).reshape(P, -1))

    ropeq_t = tile_rows(ropeq, TT_N).astype(cdt)
    ropek_t = tile_rows(ropek, TT_N).astype(cdt)

    x = np.asarray(x, np.float32)
    in_maps = []
    for core in range(N_CORES):
        b, g = core // NKV, core % NKV
        xb = x[b]                                    # [T, C]
        xt = xb.reshape(NCH, TCH, CT_N, P).transpose(0, 3, 2, 1)
        xt = np.ascontiguousarray(xt).reshape(NCH * P, CT_N * TCH)
        wqkv = np.concatenate([wq[:, g * GD:(g + 1) * GD],
                               wk[:, g * D:(g + 1) * D],
                               wv[:, g * D:(g + 1) * D]], axis=1)
        wqkv = np.asarray(wqkv, np.float32)
        wg = np.asarray(w_gate[:, g * GD:(g + 1) * GD], np.float32)
        wp = np.asarray(w_proj[g * GD:(g + 1) * GD, :], np.float32)
        in_maps.append({
            "xt": xt.astype(cdt),
            "ident": np.eye(P, dtype=np.float32).astype(cdt),
            "wqkv": tile_rows(wqkv, CT_N).astype(cdt),
            "wgate": tile_rows(wg, CT_N).astype(cdt),
            "wproj": tile_rows(wp, HG).astype(cdt),
            "ropeq": ropeq_t,
            "ropek": ropek_t,
        })
    return in_maps, rope_blocks


def kernel(x, cos, sin, wq, wk, wv, w_gate, w_proj, q_norm_w, k_norm_w):
    from concourse.bass_utils import run_bass_kernel_spmd

    x = np.asarray(x)
    in_maps, rope_blocks = make_core_inputs(
        x, cos, sin, wq, wk, wv, w_gate, w_proj, q_norm_w, k_norm_w)
    nc = _build_nc(DT_MODE, rope_blocks=rope_blocks)
    res = run_bass_kernel_spmd(nc, in_maps, list(range(N_CORES)))
    partial = np.stack([res.results[i]["out"] for i in range(N_CORES)])
    out = partial.reshape(B, NKV, T, C).sum(axis=1)
    return out.astype(np.float32)
